# revision 28
# baseline (speedup 1.0000x reference)
"""Causal multi-head attention (B=2, S=2048, D=1024, H=16) on 8 trn2 NeuronCores.

Sharding: data-parallel over batch (2 groups of 4 cores), tensor-parallel over
heads within a group (4 heads/core). Each core computes qkv projection for its
head slice, causal flash-style attention, and a partial output projection;
the host sums the 4 partials per batch element (partials land as bf16 and are
upcast+summed in f32 on host).

Device-side layout notes (per core):
  - x arrives pre-transposed (host does x[b].T, cast bf16): xT [D=1024, S=2048].
  - q,k are produced transposed (qT/kT [head_dim, S]) so score matmuls
    contract head_dim on partitions: scores^T [Sk, Sq] = kT_tile^T @ qT.
    Head pairs sit at partition bases 0/64, so their K=64 score matmuls run
    concurrently in distinct PE row groups.
  - softmax denominators come from a ones-column appended to V (M=65 in the
    PV matmul). Normalization never touches DRAM: a fast-approx reciprocal
    runs on the denominator row (partition 64) and is broadcast down the
    partitions by a tiny K=1 fp32r matmul into PSUM; one DVE multiply per
    head then scales the staged attention output into the projection lhsT.
  - causal masking zeroes the k>q triangle of exp(scores) on GPSIMD; exp
    needs no max-subtraction (|score| < ~4). Diagonal tiles restrict all
    work (matmul N, exp, mask) to the valid column range.
  - everything is emitted interleaved per q-chunk (projection c, attention c,
    output-projection c-1) so ACT exp work overlaps PE projection work. The
    ACT engine issues no DMAs mid-kernel (exp only); outputs go on the sync
    queue. Input loads are sliced along the contraction dim across both the
    sync and scalar queues so the first projection matmul starts as soon as
    ~384KB has landed; dummy matmuls warm the PE clock during the load.
"""

import numpy as np
import ml_dtypes

import concourse.bass as bass
import concourse.mybir as mybir
import concourse.tile as tile
from concourse import bacc
from concourse.bass_utils import run_bass_kernel_spmd
from concourse.masks import make_identity

B, S, D, H = 2, 2048, 1024, 16
DH = D // H              # 64
HL = 4                   # heads per core
N_CORES = 8
KO = D // 128            # 8 contraction subtiles for the qkv projection
CH = 512                 # q chunk (matmul moving dim)
NCH = S // CH            # 4
KT = S // 128            # 16 k tiles
VW = DH + 1              # v columns incl the ones column
F32 = mybir.dt.float32
F32R = mybir.dt.float32r
BF16 = mybir.dt.bfloat16
EXP = mybir.ActivationFunctionType.Exp

_cached = {}


def build():
    if "nc" in _cached:
        return _cached["nc"]

    nc = bacc.Bacc("TRN2", target_bir_lowering=False, debug=False)

    xT = nc.dram_tensor("xT", [D, S], BF16, kind="ExternalInput")
    wq = nc.dram_tensor("wq", [D, HL * DH], BF16, kind="ExternalInput")
    wk = nc.dram_tensor("wk", [D, HL * DH], BF16, kind="ExternalInput")
    wv = nc.dram_tensor("wv", [D, HL * DH], BF16, kind="ExternalInput")
    wo = nc.dram_tensor("wo", [HL * DH, D], BF16, kind="ExternalInput")
    out = nc.dram_tensor("out", [S, D], BF16, kind="ExternalOutput")

    xT_v = xT[:].rearrange("(ko p) s -> p ko s", p=128)      # [128, 8, 2048]
    wq_v = wq[:].rearrange("(ko p) m -> p ko m", p=128)      # [128, 8, 256]
    wk_v = wk[:].rearrange("(ko p) m -> p ko m", p=128)
    wv_v = wv[:].rearrange("(ko p) m -> p ko m", p=128)
    wo_v = wo[:].rearrange("(ko p) n -> p ko n", p=128)      # [128, 2, 1024]
    out_v = out[:].rearrange("(t p) n -> p t n", p=128)      # [128, 16, 1024]

    with tile.TileContext(nc) as tc:
        with (
            tc.tile_pool(name="persist", bufs=1) as pp,
            tc.tile_pool(name="mm", bufs=2, space=bass.MemorySpace.PSUM) as mmp,
            tc.tile_pool(name="ps2s", bufs=2, space=bass.MemorySpace.PSUM) as ps2s,
            tc.tile_pool(name="ps2o", bufs=1, space=bass.MemorySpace.PSUM) as ps2o,
            tc.tile_pool(name="ptp", bufs=3) as ptp,
            tc.tile_pool(name="stg", bufs=3) as stg,
            tc.tile_pool(name="ostg", bufs=4) as ostg,
            tc.tile_pool(name="tkp", bufs=1) as tkp,
        ):
            # per-chunk tiles so later phases start as soon as inputs land
            xT_sb = [pp.tile([128, KO, CH], BF16, tag=f"xT{c}", name=f"xT{c}")
                     for c in range(NCH)]
            wq_sb = pp.tile([128, KO, HL * DH], BF16, tag="wq")
            wk_sb = pp.tile([128, KO, HL * DH], BF16, tag="wk")
            wv_sb = pp.tile([128, KO, HL * DH], BF16, tag="wv")
            wo_sb = pp.tile([128, 2, D], BF16, tag="wo")
            qT_sb = [[pp.tile([128, CH], BF16, tag=f"qT{m}{c}", name=f"qT{m}{c}")
                      for c in range(NCH)] for m in range(2)]
            kT_sb = [[pp.tile([128, CH], BF16, tag=f"kT{m}{c}", name=f"kT{m}{c}")
                      for c in range(NCH)] for m in range(2)]
            # v with a ones column appended per head: [v_h (64) | 1]
            v_sb = [pp.tile([128, HL * VW], BF16, tag=f"v{t}", name=f"v{t}")
                    for t in range(KT)]
            # normalized attention output, transposed: proj lhsT, per q-chunk
            pj_sb = [[pp.tile([128, CH], BF16, tag=f"pj{c}{p}", name=f"pj{c}{p}")
                      for p in range(2)] for c in range(NCH)]

            # input loads, interleaved across the sync and scalar DMA queues
            # in first-use order; the first projection matmul only needs the
            # ko=0/1 slices of wk and xT[0], and xT[0] is split across both
            # queues so the k-projection is never starved.
            nc.scalar.dma_start(wk_sb[:, 0:2, :], wk_v[:, 0:2, :])
            nc.sync.dma_start(xT_sb[0][:, 0:2, :], xT_v[:, 0:2, 0:CH])
            nc.scalar.dma_start(wk_sb[:, 2:4, :], wk_v[:, 2:4, :])
            nc.sync.dma_start(xT_sb[0][:, 2:4, :], xT_v[:, 2:4, 0:CH])
            nc.scalar.dma_start(wk_sb[:, 4:8, :], wk_v[:, 4:8, :])
            nc.scalar.dma_start(xT_sb[0][:, 4:6, :], xT_v[:, 4:6, 0:CH])
            nc.sync.dma_start(wq_sb[:, 0:4, :], wq_v[:, 0:4, :])
            nc.scalar.dma_start(xT_sb[0][:, 6:8, :], xT_v[:, 6:8, 0:CH])
            nc.sync.dma_start(wq_sb[:, 4:8, :], wq_v[:, 4:8, :])
            nc.scalar.dma_start(wv_sb[:, 0:4, :], wv_v[:, 0:4, :])
            nc.sync.dma_start(wv_sb[:, 4:8, :], wv_v[:, 4:8, :])
            nc.scalar.dma_start(wo_sb[:], wo_v)
            nc.sync.dma_start(xT_sb[1][:, 0:4, :], xT_v[:, 0:4, CH:2 * CH])
            nc.sync.dma_start(xT_sb[1][:, 4:8, :], xT_v[:, 4:8, CH:2 * CH])
            nc.sync.dma_start(xT_sb[2][:], xT_v[:, :, 2 * CH:3 * CH])
            nc.scalar.dma_start(xT_sb[3][:], xT_v[:, :, 3 * CH:4 * CH])

            # scratch for PE warm-up matmuls: first memset on the gpsimd
            # queue so the warm-ups can issue as early as possible
            wsc = pp.tile([128, 256], BF16, tag="wsc")
            nc.gpsimd.memset(wsc[:], 1.0)
            bias0 = pp.tile([128, 1], F32, tag="bias0")
            nc.gpsimd.memset(bias0[:], 0.0)
            # head-3 rows of wo at partitions 0..63: the tail projection
            # contracts per-head (K=64) so the odd head needs no shift DMA
            wo_t3 = pp.tile([64, D], BF16, tag="wo_t3")
            nc.scalar.dma_start(wo_t3[:], wo_v[64:128, 1, :])
            # bf16 identity: folds the prepass stash into the tail psum via
            # an accumulating matmul, so the tail needs no DVE adds
            idb = pp.tile([128, 128], BF16, tag="idb")
            make_identity(nc, idb[:])
            # reciprocal-broadcast operands, all at partition base 0 (the
            # custom-DVE reciprocal mis-executes at partition offsets): the
            # denominator row is DMA-shifted from partition 64 to 0, then a
            # K=64 matmul whose weight column is 1 at partition 0 and 0
            # elsewhere broadcasts 1/l down 64 psum partitions.
            ones_r = pp.tile([128, DH], BF16, tag="ones_r")
            nc.gpsimd.memset(ones_r[:], 0.0)
            nc.gpsimd.memset(ones_r[0:1, :], 1.0)
            rin_t = [pp.tile([1, CH], F32, tag=f"rin{hh}", name=f"rin{hh}")
                     for hh in range(2)]
            rc_t = [pp.tile([1, CH], F32, tag=f"rc{hh}", name=f"rc{hh}")
                    for hh in range(2)]
            rcb_t = [pp.tile([64, CH], BF16, tag=f"rcb{hh}", name=f"rcb{hh}")
                     for hh in range(2)]
            for hh in range(2):
                nc.gpsimd.memset(rcb_t[hh][:], 0.0)
            for t in range(KT):
                nc.gpsimd.memset(v_sb[t][:], 1.0)

            # dummy matmuls on the scratch tile: keep the PE busy from ~7us
            # through the input-load window so the HAM clock gate is released
            # (K=8/8) before and during the first real projection matmuls.
            wps = mmp.tile([128, CH], F32, tag="mm", name="mm")
            for _ in range(16):
                nc.tensor.matmul(
                    wps[:, 0:256],
                    lhsT=wsc[:, 0:128],
                    rhs=wsc[:],
                    start=True, stop=True,
                )

            def p1_groups(c):
                """qT, kT, v projection psum-groups for chunk c (injectable).
                For chunk 0 (paced by the input DMA) the two m-groups of each
                of k and q are interleaved per ko-slice across both psum
                buffers, so each arriving slice unlocks twice the matmuls."""
                groups = []
                for wsb, dst in ((wk_sb, kT_sb), (wq_sb, qT_sb)):
                    if c == 0:
                        def g(wsb=wsb, dst=dst):
                            ps = [mmp.tile([128, CH], F32, tag="mm", name="mm")
                                  for _ in range(2)]
                            for ko in range(KO):
                                for m in range(2):
                                    nc.tensor.matmul(
                                        ps[m],
                                        lhsT=wsb[:, ko, m * 128:(m + 1) * 128],
                                        rhs=xT_sb[c][:, ko, :],
                                        start=(ko == 0),
                                        stop=(ko == KO - 1),
                                    )
                            for m in range(2):
                                nc.vector.tensor_copy(dst[m][c][:], ps[m])
                        groups.append(g)
                        continue
                    for m in range(2):          # 128-col tiles (2 heads each)
                        def g(wsb=wsb, dst=dst, m=m):
                            ps = mmp.tile([128, CH], F32, tag="mm", name="mm")
                            for ko in range(KO):
                                nc.tensor.matmul(
                                    ps,
                                    lhsT=wsb[:, ko, m * 128:(m + 1) * 128],
                                    rhs=xT_sb[c][:, ko, :],
                                    start=(ko == 0),
                                    stop=(ko == KO - 1),
                                )
                            nc.vector.tensor_copy(dst[m][c][:], ps)
                        groups.append(g)
                for tt in range(4):             # v tiles of this chunk
                    def g(tt=tt):
                        t = 4 * c + tt
                        ps = mmp.tile([128, CH], F32, tag="mm", name="mm")
                        for ko in range(KO):
                            nc.tensor.matmul(
                                ps[:, :HL * DH],
                                lhsT=xT_sb[c][:, ko, tt * 128:(tt + 1) * 128],
                                rhs=wv_sb[:, ko, :],
                                start=(ko == 0),
                                stop=(ko == KO - 1),
                            )
                        dst = v_sb[t][:].rearrange("p (h e) -> p h e", e=VW)
                        src = ps[:, :HL * DH].rearrange("p (h e) -> p h e", e=DH)
                        nc.vector.tensor_copy(dst[:, :, :DH], src)
                    groups.append(g)
                return groups

            def attention(c, pr, first=(), inject=()):
                """Causal attention for q-chunk c, head pair pr (2pr, 2pr+1).
                `first` emitters run at t=0 (before the first PV matmul);
                `inject` emitters are spread between t-steps so their PE work
                fills the gaps of this ACT-bound stretch. Returns the SBUF
                staging copies of the two PV psums ([65, CH]: row 64 is the
                softmax denominator)."""
                first = list(first)
                inject = list(inject)
                nk = 4 * c + 4                  # k tiles this chunk needs
                every = max(1, (nk + len(inject) - 1) // max(1, len(inject))) \
                    if inject else 0
                po = [ps2o.tile([65, CH], F32, tag=f"po{hh}", name=f"po{hh}")
                      for hh in range(2)]

                def emit_score(t):
                    """score pair matmuls + exp + causal triangle mask."""
                    d = t - 4 * c               # >= 0 on diagonal tiles
                    lo = 128 * max(d, 0)        # first valid column in chunk
                    st = ps2s.tile([128, 2, CH], F32, tag="s", name="s")
                    for hh in range(2):
                        nc.tensor.matmul(
                            st[:, hh, lo:],
                            lhsT=kT_sb[pr][t // 4][
                                hh * 64:hh * 64 + 64,
                                (t % 4) * 128:(t % 4) * 128 + 128],
                            rhs=qT_sb[pr][c][hh * 64:hh * 64 + 64, lo:],
                            start=True,
                            stop=True,
                        )
                    pt = ptp.tile([128, 2, CH], BF16, tag="pt", name="pt")
                    nc.scalar.activation(
                        pt[:, :, lo:], st[:, :, lo:], EXP,
                        bias=bias0[:, 0:1],
                        scale=float(DH) ** -0.5,
                    )
                    if d >= 0:                  # zero the k>q triangle, which
                        # only spans the first 128 columns of the valid range
                        nc.gpsimd.affine_select(
                            out=pt[:, :, lo:lo + 128],
                            in_=pt[:, :, lo:lo + 128],
                            compare_op=mybir.AluOpType.is_ge,
                            fill=0.0,
                            base=0,
                            pattern=[[0, 2], [1, 128]],
                            channel_multiplier=-1,
                        )
                    return pt, lo

                # software pipeline: scores run one t ahead of the PV matmuls
                # so the PE never sits behind the exp of the tile it consumes
                pts = {0: emit_score(0)}
                for t in range(nk):
                    if t + 1 < nk:
                        pts[t + 1] = emit_score(t + 1)
                    if t == 0:
                        for g in first:
                            g()
                    # injected PE work lands between the look-ahead score and
                    # this step's PV matmuls, covering the exp/mask wait
                    if inject and t % every == every - 1:
                        inject.pop(0)()
                    pt, lo = pts.pop(t)
                    for hh in range(2):
                        h = 2 * pr + hh
                        nc.tensor.matmul(
                            po[hh][:, lo:],
                            lhsT=v_sb[t][:, h * VW:(h + 1) * VW],
                            rhs=pt[:, hh, lo:],
                            start=(t == 0),
                            stop=(t == nk - 1),
                        )
                for g in inject:
                    g()
                # stage po to SBUF (frees the psum bank for the next pair)
                # and immediately DMA the denominator row to partition 0, so
                # the shift is in flight while the stretch drains; fin_a then
                # only runs reciprocal+cast.
                ots = []
                for hh in range(2):
                    ot = stg.tile([65, CH], F32, tag=f"ot{pr}{hh}",
                                  name=f"ot{pr}{hh}")
                    nc.vector.tensor_copy(ot[:], po[hh][:])
                    nc.sync.dma_start(rin_t[hh][0:1, :], ot[64:65, :])
                    ots.append(ot)
                return ots

            def fin_a(c, pr, ots):
                """normalization prefix, no PE ops: ~18-bit reciprocal of the
                shifted denominator row, bf16 downcast. Inject at the start of
                the next stretch so the chain completes before fin_b's
                matmuls reach the head of the PE queue."""
                for hh in (1, 0):
                    nc.vector.reciprocal_approx_fast(rc_t[hh][0:1, :],
                                                     rin_t[hh][0:1, :])
                    nc.vector.tensor_copy(rcb_t[hh][0:1, :],
                                          rc_t[hh][0:1, :])

            def fin_b(c, pr, ots):
                """normalization suffix: K=64 broadcast matmul of 1/l down 64
                psum partitions, then one DVE multiply per head into the
                projection lhsT. The odd head lands via one SBUF shift DMA."""
                for hh in (1, 0):
                    rep = mmp.tile([128, CH], F32, tag="mm", name="mm")
                    nc.tensor.matmul(
                        rep[0:64, :],
                        lhsT=ones_r[0:64, :],
                        rhs=rcb_t[hh][0:64, :],
                        start=True, stop=True,
                    )
                    if hh == 0:
                        nc.vector.tensor_mul(
                            pj_sb[c][pr][0:64, :], ots[hh][0:64, :],
                            rep[0:64, :])
                    else:
                        tmp = stg.tile([64, CH], BF16, tag=f"tmp{pr}",
                                       name=f"tmp{pr}")
                        nc.vector.tensor_mul(tmp, ots[hh][0:64, :],
                                             rep[0:64, :])
                        nc.sync.dma_start(pj_sb[c][pr][64:128, :], tmp)

            def p3_groups(c, ko_list=(0, 1)):
                """partial output projection psum-groups for chunk c over the
                given pj pairs; output staged as bf16, DMA'd on sync."""
                groups = []
                for tt in range(4):
                    for n2 in range(2):
                        def g(tt=tt, n2=n2):
                            mt = 4 * c + tt
                            prt = mmp.tile([128, CH], F32, tag="mm", name="mm")
                            for j, ko in enumerate(ko_list):
                                nc.tensor.matmul(
                                    prt,
                                    lhsT=pj_sb[c][ko][:, tt * 128:tt * 128 + 128],
                                    rhs=wo_sb[:, ko, n2 * CH:(n2 + 1) * CH],
                                    start=(j == 0),
                                    stop=(j == len(ko_list) - 1),
                                )
                            ob = ostg.tile([128, CH], BF16, tag="ob", name="ob")
                            nc.vector.tensor_copy(ob, prt)
                            nc.sync.dma_start(
                                out_v[:, mt, n2 * CH:(n2 + 1) * CH], ob)
                        groups.append(g)
                return groups

            pending = {}
            tks = {}
            lc = NCH - 1

            def prepass_groups():
                """ko=0 (pair 0) half of the last chunk's projection, stashed
                in SBUF tk tiles; runs inside the last attention block."""
                groups = []
                for tt in range(4):
                    for n2 in range(2):
                        def g(tt=tt, n2=n2):
                            prt = mmp.tile([128, CH], F32, tag="mm", name="mm")
                            nc.tensor.matmul(
                                prt,
                                lhsT=pj_sb[lc][0][:, tt * 128:tt * 128 + 128],
                                rhs=wo_sb[:, 0, n2 * CH:(n2 + 1) * CH],
                                start=True, stop=True,
                            )
                            tk = tkp.tile([128, CH], BF16, tag=f"tk{tt}{n2}",
                                          name=f"tk{tt}{n2}")
                            nc.vector.tensor_copy(tk, prt)
                            tks[(tt, n2)] = tk
                        groups.append(g)
                return groups

            def mid_insert(lst, g):
                """insert g past the middle of an inject list (so the PE
                reaches its matmuls only after the fin_a chain completed)."""
                lst.insert((2 * len(lst) + 2) // 3, g)

            for g in p1_groups(0):
                g()
            for c in range(NCH):
                first0, first1 = [], []
                inj0, inj1 = [], []
                if c > 0:
                    first0.append(
                        lambda c=c: fin_a(c - 1, 1, pending[(c - 1, 1)]))
                if c + 1 < NCH:
                    p1 = p1_groups(c + 1)
                    inj0 += p1[:4]
                    inj1 += p1[4:]
                if c > 0:
                    inj1 += p3_groups(c - 1)
                    mid_insert(inj0,
                               lambda c=c: fin_b(c - 1, 1, pending[(c - 1, 1)]))
                r0 = attention(c, 0, first0, inj0)
                first1.append(lambda c=c, r0=r0: fin_a(c, 0, r0))
                if c == 0:
                    inj1.append(lambda c=c, r0=r0: fin_b(c, 0, r0))
                else:
                    mid_insert(inj1, lambda c=c, r0=r0: fin_b(c, 0, r0))
                r1 = attention(c, 1, first1, inj1)
                pending[(c, 0)] = r0
                pending[(c, 1)] = r1

            # ---- tail: finish pair 1 of the last chunk; the prepass (ko=0
            # half of its projection, 8 matmuls) covers the fin chain latency
            # and keeps the PE clock warm. The pair-1 half contracts per-head
            # (K=64) against wo_sb rows 0:64 / the preloaded wo_t3, so the
            # odd head needs no shift DMA; the stash-adds alternate DVE and
            # GPSIMD and the flush alternates the sync and scalar queues.
            ots1 = pending[(lc, 1)]
            fin_a(lc, 1, ots1)
            for g in prepass_groups():
                g()
            tb_t = {}
            for hh in (1, 0):
                rep = mmp.tile([128, CH], F32, tag="mm", name="mm")
                nc.tensor.matmul(
                    rep[0:64, :],
                    lhsT=ones_r[0:64, :],
                    rhs=rcb_t[hh][0:64, :],
                    start=True, stop=True,
                )
                tb = stg.tile([64, CH], BF16, tag=f"tb{hh}", name=f"tb{hh}")
                nc.vector.tensor_mul(tb, ots1[hh][0:64, :], rep[0:64, :])
                tb_t[hh] = tb
            for tt in range(4):
                for n2 in range(2):
                    prt = mmp.tile([128, CH], F32, tag="mm", name="mm")
                    nc.tensor.matmul(
                        prt,
                        lhsT=tb_t[0][:, tt * 128:tt * 128 + 128],
                        rhs=wo_sb[0:64, 1, n2 * CH:(n2 + 1) * CH],
                        start=True, stop=False,
                    )
                    nc.tensor.matmul(
                        prt,
                        lhsT=tb_t[1][:, tt * 128:tt * 128 + 128],
                        rhs=wo_t3[:, n2 * CH:(n2 + 1) * CH],
                        start=False, stop=False,
                    )
                    nc.tensor.matmul(
                        prt,
                        lhsT=idb[:],
                        rhs=tks[(tt, n2)][:],
                        start=False, stop=True,
                    )
                    ob = ostg.tile([128, CH], BF16, tag="ob", name="ob")
                    # ACT is idle in the tail; DVE is not
                    nc.scalar.copy(ob, prt)
                    nc.sync.dma_start(
                        out_v[:, 4 * lc + tt, n2 * CH:(n2 + 1) * CH], ob)

    nc.compile()
    _cached["nc"] = nc
    return nc


def make_in_maps(x, w_qkv, w_out):
    bf = ml_dtypes.bfloat16
    in_maps = []
    for core in range(N_CORES):
        b, h0 = core // 4, (core % 4) * HL
        c0 = h0 * DH
        in_maps.append({
            "xT": np.ascontiguousarray(x[b].T).astype(bf),
            "wq": w_qkv[:, c0:c0 + HL * DH].astype(bf),
            "wk": w_qkv[:, D + c0:D + c0 + HL * DH].astype(bf),
            "wv": w_qkv[:, 2 * D + c0:2 * D + c0 + HL * DH].astype(bf),
            "wo": w_out[c0:c0 + HL * DH, :].astype(bf),
        })
    return in_maps


def run_sharded(x, w_qkv, w_out, trace=False, tmpdir=None):
    nc = build()
    res = run_bass_kernel_spmd(
        nc, make_in_maps(x, w_qkv, w_out), core_ids=list(range(N_CORES)),
        trace=trace, tmpdir=tmpdir,
    )
    out = np.zeros((B, S, D), np.float32)
    for core in range(N_CORES):
        out[core // 4] += res.results[core]["out"].astype(np.float32)
    return out, res.exec_time_ns


def kernel(x, w_qkv, w_out):
    out, _ = run_sharded(x, w_qkv, w_out)
    return out


# revision 31
# speedup vs baseline: 1.0074x; 1.0074x over previous
"""Causal multi-head attention (B=2, S=2048, D=1024, H=16) on 8 trn2 NeuronCores.

Sharding: data-parallel over batch (2 groups of 4 cores), tensor-parallel over
heads within a group (4 heads/core). Each core computes qkv projection for its
head slice, causal flash-style attention, and a partial output projection;
the host sums the 4 partials per batch element (partials land as bf16 and are
upcast+summed in f32 on host).

Device-side layout notes (per core):
  - x arrives pre-transposed (host does x[b].T, cast bf16): xT [D=1024, S=2048].
  - q,k are produced transposed (qT/kT [head_dim, S]) so score matmuls
    contract head_dim on partitions: scores^T [Sk, Sq] = kT_tile^T @ qT.
    Head pairs sit at partition bases 0/64, so their K=64 score matmuls run
    concurrently in distinct PE row groups.
  - softmax denominators come from a ones-column appended to V (M=65 in the
    PV matmul). Normalization never touches DRAM: a fast-approx reciprocal
    runs on the denominator row (partition 64) and is broadcast down the
    partitions by a tiny K=1 fp32r matmul into PSUM; one DVE multiply per
    head then scales the staged attention output into the projection lhsT.
  - causal masking zeroes the k>q triangle of exp(scores) on GPSIMD; exp
    needs no max-subtraction (|score| < ~4). Diagonal tiles restrict all
    work (matmul N, exp, mask) to the valid column range.
  - everything is emitted interleaved per q-chunk (projection c, attention c,
    output-projection c-1) so ACT exp work overlaps PE projection work. The
    ACT engine issues no DMAs mid-kernel (exp only); outputs go on the sync
    queue. Input loads are sliced along the contraction dim across both the
    sync and scalar queues so the first projection matmul starts as soon as
    ~384KB has landed; dummy matmuls warm the PE clock during the load.
"""

import numpy as np
import ml_dtypes

import concourse.bass as bass
import concourse.mybir as mybir
import concourse.tile as tile
from concourse import bacc
from concourse.bass_utils import run_bass_kernel_spmd
from concourse.masks import make_identity

B, S, D, H = 2, 2048, 1024, 16
DH = D // H              # 64
HL = 4                   # heads per core
N_CORES = 8
KO = D // 128            # 8 contraction subtiles for the qkv projection
CH = 512                 # q chunk (matmul moving dim)
NCH = S // CH            # 4
KT = S // 128            # 16 k tiles
VW = DH + 1              # v columns incl the ones column
F32 = mybir.dt.float32
F32R = mybir.dt.float32r
BF16 = mybir.dt.bfloat16
EXP = mybir.ActivationFunctionType.Exp

_cached = {}


def build():
    if "nc" in _cached:
        return _cached["nc"]

    nc = bacc.Bacc("TRN2", target_bir_lowering=False, debug=False)

    xT = nc.dram_tensor("xT", [D, S], BF16, kind="ExternalInput")
    wq = nc.dram_tensor("wq", [D, HL * DH], BF16, kind="ExternalInput")
    wk = nc.dram_tensor("wk", [D, HL * DH], BF16, kind="ExternalInput")
    wv = nc.dram_tensor("wv", [D, HL * DH], BF16, kind="ExternalInput")
    wo = nc.dram_tensor("wo", [HL * DH, D], BF16, kind="ExternalInput")
    out = nc.dram_tensor("out", [S, D], BF16, kind="ExternalOutput")

    xT_v = xT[:].rearrange("(ko p) s -> p ko s", p=128)      # [128, 8, 2048]
    wq_v = wq[:].rearrange("(ko p) m -> p ko m", p=128)      # [128, 8, 256]
    wk_v = wk[:].rearrange("(ko p) m -> p ko m", p=128)
    wv_v = wv[:].rearrange("(ko p) m -> p ko m", p=128)
    wo_v = wo[:].rearrange("(ko p) n -> p ko n", p=128)      # [128, 2, 1024]
    out_v = out[:].rearrange("(t p) n -> p t n", p=128)      # [128, 16, 1024]

    with tile.TileContext(nc) as tc:
        with (
            tc.tile_pool(name="persist", bufs=1) as pp,
            tc.tile_pool(name="mm", bufs=2, space=bass.MemorySpace.PSUM) as mmp,
            tc.tile_pool(name="ps2s", bufs=2, space=bass.MemorySpace.PSUM) as ps2s,
            tc.tile_pool(name="ps2o", bufs=1, space=bass.MemorySpace.PSUM) as ps2o,
            tc.tile_pool(name="ptp", bufs=3) as ptp,
            tc.tile_pool(name="stg", bufs=3) as stg,
            tc.tile_pool(name="ostg", bufs=4) as ostg,
            tc.tile_pool(name="tkp", bufs=1) as tkp,
        ):
            # per-chunk tiles so later phases start as soon as inputs land
            xT_sb = [pp.tile([128, KO, CH], BF16, tag=f"xT{c}", name=f"xT{c}")
                     for c in range(NCH)]
            wq_sb = pp.tile([128, KO, HL * DH], BF16, tag="wq")
            wk_sb = pp.tile([128, KO, HL * DH], BF16, tag="wk")
            wv_sb = pp.tile([128, KO, HL * DH], BF16, tag="wv")
            wo_sb = pp.tile([128, 2, D], BF16, tag="wo")
            qT_sb = [[pp.tile([128, CH], BF16, tag=f"qT{m}{c}", name=f"qT{m}{c}")
                      for c in range(NCH)] for m in range(2)]
            kT_sb = [[pp.tile([128, CH], BF16, tag=f"kT{m}{c}", name=f"kT{m}{c}")
                      for c in range(NCH)] for m in range(2)]
            # v with a ones column appended per head: [v_h (64) | 1]
            v_sb = [pp.tile([128, HL * VW], BF16, tag=f"v{t}", name=f"v{t}")
                    for t in range(KT)]
            # normalized attention output, transposed: proj lhsT, per q-chunk
            pj_sb = [[pp.tile([128, CH], BF16, tag=f"pj{c}{p}", name=f"pj{c}{p}")
                      for p in range(2)] for c in range(NCH)]

            # input loads, interleaved across the sync and scalar DMA queues
            # in first-use order; the first projection matmul only needs the
            # ko=0/1 slices of wk and xT[0], and xT[0] is split across both
            # queues so the k-projection is never starved.
            nc.scalar.dma_start(wk_sb[:, 0:2, :], wk_v[:, 0:2, :])
            nc.sync.dma_start(xT_sb[0][:, 0:2, :], xT_v[:, 0:2, 0:CH])
            nc.scalar.dma_start(wk_sb[:, 2:4, :], wk_v[:, 2:4, :])
            nc.sync.dma_start(xT_sb[0][:, 2:4, :], xT_v[:, 2:4, 0:CH])
            nc.scalar.dma_start(wk_sb[:, 4:8, :], wk_v[:, 4:8, :])
            nc.sync.dma_start(wq_sb[:], wq_v)
            nc.scalar.dma_start(xT_sb[0][:, 4:6, :], xT_v[:, 4:6, 0:CH])
            nc.scalar.dma_start(xT_sb[0][:, 6:8, :], xT_v[:, 6:8, 0:CH])
            nc.scalar.dma_start(wv_sb[:], wv_v)
            nc.sync.dma_start(xT_sb[1][:, 0:4, :], xT_v[:, 0:4, CH:2 * CH])
            nc.scalar.dma_start(wo_sb[:], wo_v)
            nc.sync.dma_start(xT_sb[1][:, 4:8, :], xT_v[:, 4:8, CH:2 * CH])
            nc.sync.dma_start(xT_sb[2][:], xT_v[:, :, 2 * CH:3 * CH])
            nc.scalar.dma_start(xT_sb[3][:], xT_v[:, :, 3 * CH:4 * CH])

            # scratch for PE warm-up matmuls: first memset on the gpsimd
            # queue so the warm-ups can issue as early as possible
            wsc = pp.tile([128, 256], BF16, tag="wsc")
            nc.gpsimd.memset(wsc[:], 1.0)
            bias0 = pp.tile([128, 1], F32, tag="bias0")
            nc.gpsimd.memset(bias0[:], 0.0)
            # head-3 rows of wo at partitions 0..63: the tail projection
            # contracts per-head (K=64) so the odd head needs no shift DMA
            wo_t3 = pp.tile([64, D], BF16, tag="wo_t3")
            nc.scalar.dma_start(wo_t3[:], wo_v[64:128, 1, :])
            # bf16 identity: folds the prepass stash into the tail psum via
            # an accumulating matmul, so the tail needs no DVE adds
            idb = pp.tile([128, 128], BF16, tag="idb")
            make_identity(nc, idb[:])
            # reciprocal-broadcast operands, all at partition base 0 (the
            # custom-DVE reciprocal mis-executes at partition offsets): the
            # denominator row is DMA-shifted from partition 64 to 0, then a
            # K=64 matmul whose weight column is 1 at partition 0 and 0
            # elsewhere broadcasts 1/l down 64 psum partitions.
            ones_r = pp.tile([128, DH], BF16, tag="ones_r")
            nc.gpsimd.memset(ones_r[:], 0.0)
            nc.gpsimd.memset(ones_r[0:1, :], 1.0)
            rin_t = [pp.tile([1, CH], F32, tag=f"rin{hh}", name=f"rin{hh}")
                     for hh in range(2)]
            rc_t = [pp.tile([1, CH], F32, tag=f"rc{hh}", name=f"rc{hh}")
                    for hh in range(2)]
            rcb_t = [pp.tile([64, CH], BF16, tag=f"rcb{hh}", name=f"rcb{hh}")
                     for hh in range(2)]
            for hh in range(2):
                nc.gpsimd.memset(rcb_t[hh][:], 0.0)
            for t in range(KT):
                nc.gpsimd.memset(v_sb[t][:], 1.0)

            # dummy matmuls on the scratch tile: keep the PE busy from ~7us
            # through the input-load window so the HAM clock gate is released
            # (K=8/8) before and during the first real projection matmuls.
            wps = mmp.tile([128, CH], F32, tag="mm", name="mm")
            for _ in range(16):
                nc.tensor.matmul(
                    wps[:, 0:256],
                    lhsT=wsc[:, 0:128],
                    rhs=wsc[:],
                    start=True, stop=True,
                )

            def p1_groups(c):
                """qT, kT, v projection psum-groups for chunk c (injectable).
                For chunk 0 (paced by the input DMA) the two m-groups of each
                of k and q are interleaved per ko-slice across both psum
                buffers, so each arriving slice unlocks twice the matmuls."""
                groups = []
                for wsb, dst in ((wk_sb, kT_sb), (wq_sb, qT_sb)):
                    if c == 0:
                        def g(wsb=wsb, dst=dst):
                            ps = [mmp.tile([128, CH], F32, tag="mm", name="mm")
                                  for _ in range(2)]
                            for ko in range(KO):
                                for m in range(2):
                                    nc.tensor.matmul(
                                        ps[m],
                                        lhsT=wsb[:, ko, m * 128:(m + 1) * 128],
                                        rhs=xT_sb[c][:, ko, :],
                                        start=(ko == 0),
                                        stop=(ko == KO - 1),
                                    )
                            for m in range(2):
                                nc.vector.tensor_copy(dst[m][c][:], ps[m])
                        groups.append(g)
                        continue
                    for m in range(2):          # 128-col tiles (2 heads each)
                        def g(wsb=wsb, dst=dst, m=m):
                            ps = mmp.tile([128, CH], F32, tag="mm", name="mm")
                            for ko in range(KO):
                                nc.tensor.matmul(
                                    ps,
                                    lhsT=wsb[:, ko, m * 128:(m + 1) * 128],
                                    rhs=xT_sb[c][:, ko, :],
                                    start=(ko == 0),
                                    stop=(ko == KO - 1),
                                )
                            nc.vector.tensor_copy(dst[m][c][:], ps)
                        groups.append(g)
                for tt in range(4):             # v tiles of this chunk
                    def g(tt=tt):
                        t = 4 * c + tt
                        ps = mmp.tile([128, CH], F32, tag="mm", name="mm")
                        for ko in range(KO):
                            nc.tensor.matmul(
                                ps[:, :HL * DH],
                                lhsT=xT_sb[c][:, ko, tt * 128:(tt + 1) * 128],
                                rhs=wv_sb[:, ko, :],
                                start=(ko == 0),
                                stop=(ko == KO - 1),
                            )
                        dst = v_sb[t][:].rearrange("p (h e) -> p h e", e=VW)
                        src = ps[:, :HL * DH].rearrange("p (h e) -> p h e", e=DH)
                        nc.vector.tensor_copy(dst[:, :, :DH], src)
                    groups.append(g)
                return groups

            def attention(c, pr, first=(), inject=()):
                """Causal attention for q-chunk c, head pair pr (2pr, 2pr+1).
                `first` emitters run at t=0 (before the first PV matmul);
                `inject` emitters are spread between t-steps so their PE work
                fills the gaps of this ACT-bound stretch. Returns the SBUF
                staging copies of the two PV psums ([65, CH]: row 64 is the
                softmax denominator)."""
                first = list(first)
                inject = list(inject)
                nk = 4 * c + 4                  # k tiles this chunk needs
                every = max(1, (nk + len(inject) - 1) // max(1, len(inject))) \
                    if inject else 0
                po = [ps2o.tile([65, CH], F32, tag=f"po{hh}", name=f"po{hh}")
                      for hh in range(2)]

                def emit_score(t):
                    """score pair matmuls + exp + causal triangle mask."""
                    d = t - 4 * c               # >= 0 on diagonal tiles
                    lo = 128 * max(d, 0)        # first valid column in chunk
                    st = ps2s.tile([128, 2, CH], F32, tag="s", name="s")
                    for hh in range(2):
                        nc.tensor.matmul(
                            st[:, hh, lo:],
                            lhsT=kT_sb[pr][t // 4][
                                hh * 64:hh * 64 + 64,
                                (t % 4) * 128:(t % 4) * 128 + 128],
                            rhs=qT_sb[pr][c][hh * 64:hh * 64 + 64, lo:],
                            start=True,
                            stop=True,
                        )
                    pt = ptp.tile([128, 2, CH], BF16, tag="pt", name="pt")
                    nc.scalar.activation(
                        pt[:, :, lo:], st[:, :, lo:], EXP,
                        bias=bias0[:, 0:1],
                        scale=float(DH) ** -0.5,
                    )
                    if d >= 0:                  # zero the k>q triangle, which
                        # only spans the first 128 columns of the valid range
                        nc.gpsimd.affine_select(
                            out=pt[:, :, lo:lo + 128],
                            in_=pt[:, :, lo:lo + 128],
                            compare_op=mybir.AluOpType.is_ge,
                            fill=0.0,
                            base=0,
                            pattern=[[0, 2], [1, 128]],
                            channel_multiplier=-1,
                        )
                    return pt, lo

                # software pipeline: scores run one t ahead of the PV matmuls
                # so the PE never sits behind the exp of the tile it consumes
                pts = {0: emit_score(0)}
                for t in range(nk):
                    if t + 1 < nk:
                        pts[t + 1] = emit_score(t + 1)
                    if t == 0:
                        for g in first:
                            g()
                    # injected PE work lands between the look-ahead score and
                    # this step's PV matmuls, covering the exp/mask wait
                    if inject and t % every == every - 1:
                        inject.pop(0)()
                    pt, lo = pts.pop(t)
                    for hh in range(2):
                        h = 2 * pr + hh
                        nc.tensor.matmul(
                            po[hh][:, lo:],
                            lhsT=v_sb[t][:, h * VW:(h + 1) * VW],
                            rhs=pt[:, hh, lo:],
                            start=(t == 0),
                            stop=(t == nk - 1),
                        )
                for g in inject:
                    g()
                # stage po to SBUF (frees the psum bank for the next pair)
                # and immediately DMA the denominator row to partition 0, so
                # the shift is in flight while the stretch drains; fin_a then
                # only runs reciprocal+cast.
                ots = []
                for hh in range(2):
                    ot = stg.tile([65, CH], F32, tag=f"ot{pr}{hh}",
                                  name=f"ot{pr}{hh}")
                    nc.vector.tensor_copy(ot[:], po[hh][:])
                    nc.sync.dma_start(rin_t[hh][0:1, :], ot[64:65, :])
                    ots.append(ot)
                return ots

            def fin_a(c, pr, ots):
                """normalization prefix, no PE ops: ~18-bit reciprocal of the
                shifted denominator row, bf16 downcast. Inject at the start of
                the next stretch so the chain completes before fin_b's
                matmuls reach the head of the PE queue."""
                for hh in (1, 0):
                    nc.vector.reciprocal_approx_fast(rc_t[hh][0:1, :],
                                                     rin_t[hh][0:1, :])
                    nc.vector.tensor_copy(rcb_t[hh][0:1, :],
                                          rc_t[hh][0:1, :])

            def fin_b(c, pr, ots):
                """normalization suffix: K=64 broadcast matmul of 1/l down 64
                psum partitions, then one DVE multiply per head into the
                projection lhsT. The odd head lands via one SBUF shift DMA."""
                for hh in (1, 0):
                    rep = mmp.tile([128, CH], F32, tag="mm", name="mm")
                    nc.tensor.matmul(
                        rep[0:64, :],
                        lhsT=ones_r[0:64, :],
                        rhs=rcb_t[hh][0:64, :],
                        start=True, stop=True,
                    )
                    if hh == 0:
                        nc.vector.tensor_mul(
                            pj_sb[c][pr][0:64, :], ots[hh][0:64, :],
                            rep[0:64, :])
                    else:
                        tmp = stg.tile([64, CH], BF16, tag=f"tmp{pr}",
                                       name=f"tmp{pr}")
                        nc.vector.tensor_mul(tmp, ots[hh][0:64, :],
                                             rep[0:64, :])
                        nc.sync.dma_start(pj_sb[c][pr][64:128, :], tmp)

            def p3_groups(c, ko_list=(0, 1)):
                """partial output projection psum-groups for chunk c over the
                given pj pairs; output staged as bf16, DMA'd on sync."""
                groups = []
                for tt in range(4):
                    for n2 in range(2):
                        def g(tt=tt, n2=n2):
                            mt = 4 * c + tt
                            prt = mmp.tile([128, CH], F32, tag="mm", name="mm")
                            for j, ko in enumerate(ko_list):
                                nc.tensor.matmul(
                                    prt,
                                    lhsT=pj_sb[c][ko][:, tt * 128:tt * 128 + 128],
                                    rhs=wo_sb[:, ko, n2 * CH:(n2 + 1) * CH],
                                    start=(j == 0),
                                    stop=(j == len(ko_list) - 1),
                                )
                            ob = ostg.tile([128, CH], BF16, tag="ob", name="ob")
                            nc.vector.tensor_copy(ob, prt)
                            nc.sync.dma_start(
                                out_v[:, mt, n2 * CH:(n2 + 1) * CH], ob)
                        groups.append(g)
                return groups

            pending = {}
            tks = {}
            lc = NCH - 1

            def prepass_groups():
                """ko=0 (pair 0) half of the last chunk's projection, stashed
                in SBUF tk tiles; runs inside the last attention block."""
                groups = []
                for tt in range(4):
                    for n2 in range(2):
                        def g(tt=tt, n2=n2):
                            prt = mmp.tile([128, CH], F32, tag="mm", name="mm")
                            nc.tensor.matmul(
                                prt,
                                lhsT=pj_sb[lc][0][:, tt * 128:tt * 128 + 128],
                                rhs=wo_sb[:, 0, n2 * CH:(n2 + 1) * CH],
                                start=True, stop=True,
                            )
                            tk = tkp.tile([128, CH], BF16, tag=f"tk{tt}{n2}",
                                          name=f"tk{tt}{n2}")
                            nc.vector.tensor_copy(tk, prt)
                            tks[(tt, n2)] = tk
                        groups.append(g)
                return groups

            def mid_insert(lst, g):
                """insert g past the middle of an inject list (so the PE
                reaches its matmuls only after the fin_a chain completed)."""
                lst.insert((2 * len(lst) + 2) // 3, g)

            for g in p1_groups(0):
                g()
            for c in range(NCH):
                first0, first1 = [], []
                inj0, inj1 = [], []
                if c > 0:
                    first0.append(
                        lambda c=c: fin_a(c - 1, 1, pending[(c - 1, 1)]))
                if c + 1 < NCH:
                    p1 = p1_groups(c + 1)
                    inj0 += p1[:4]
                    inj1 += p1[4:]
                if c > 0:
                    p3p = p3_groups(c - 1)
                    fb = lambda c=c: fin_b(c - 1, 1, pending[(c - 1, 1)])
                    if c == lc:
                        # no p1 work left: balance the output projection
                        # across both stretches of the ACT-bound last chunk.
                        # fin_b must precede the p3 groups that read its pj.
                        inj0 = [fb] + p3p[:4]
                        inj1 += p3p[4:]
                    else:
                        inj1 += p3p
                        mid_insert(inj0, fb)
                r0 = attention(c, 0, first0, inj0)
                first1.append(lambda c=c, r0=r0: fin_a(c, 0, r0))
                if c == 0:
                    inj1.append(lambda c=c, r0=r0: fin_b(c, 0, r0))
                elif c == lc:
                    inj1 += [lambda c=c, r0=r0: fin_b(c, 0, r0)]
                    inj1 += prepass_groups()
                else:
                    mid_insert(inj1, lambda c=c, r0=r0: fin_b(c, 0, r0))
                r1 = attention(c, 1, first1, inj1)
                pending[(c, 0)] = r0
                pending[(c, 1)] = r1

            # ---- tail: finish pair 1 of the last chunk; the prepass (ko=0
            # half of its projection, 8 matmuls) covers the fin chain latency
            # and keeps the PE clock warm. The pair-1 half contracts per-head
            # (K=64) against wo_sb rows 0:64 / the preloaded wo_t3, so the
            # odd head needs no shift DMA; the stash-adds alternate DVE and
            # GPSIMD and the flush alternates the sync and scalar queues.
            ots1 = pending[(lc, 1)]
            fin_a(lc, 1, ots1)
            tb_t = {}
            for hh in (1, 0):
                rep = mmp.tile([128, CH], F32, tag="mm", name="mm")
                nc.tensor.matmul(
                    rep[0:64, :],
                    lhsT=ones_r[0:64, :],
                    rhs=rcb_t[hh][0:64, :],
                    start=True, stop=True,
                )
                tb = stg.tile([64, CH], BF16, tag=f"tb{hh}", name=f"tb{hh}")
                nc.vector.tensor_mul(tb, ots1[hh][0:64, :], rep[0:64, :])
                tb_t[hh] = tb
            for tt in range(4):
                for n2 in range(2):
                    prt = mmp.tile([128, CH], F32, tag="mm", name="mm")
                    nc.tensor.matmul(
                        prt,
                        lhsT=tb_t[0][:, tt * 128:tt * 128 + 128],
                        rhs=wo_sb[0:64, 1, n2 * CH:(n2 + 1) * CH],
                        start=True, stop=False,
                    )
                    nc.tensor.matmul(
                        prt,
                        lhsT=tb_t[1][:, tt * 128:tt * 128 + 128],
                        rhs=wo_t3[:, n2 * CH:(n2 + 1) * CH],
                        start=False, stop=False,
                    )
                    nc.tensor.matmul(
                        prt,
                        lhsT=idb[:],
                        rhs=tks[(tt, n2)][:],
                        start=False, stop=True,
                    )
                    ob = ostg.tile([128, CH], BF16, tag="ob", name="ob")
                    # ACT is idle in the tail; DVE is not
                    nc.scalar.copy(ob, prt)
                    nc.sync.dma_start(
                        out_v[:, 4 * lc + tt, n2 * CH:(n2 + 1) * CH], ob)

    nc.compile()
    _cached["nc"] = nc
    return nc


def make_in_maps(x, w_qkv, w_out):
    bf = ml_dtypes.bfloat16
    in_maps = []
    for core in range(N_CORES):
        b, h0 = core // 4, (core % 4) * HL
        c0 = h0 * DH
        in_maps.append({
            "xT": np.ascontiguousarray(x[b].T).astype(bf),
            "wq": w_qkv[:, c0:c0 + HL * DH].astype(bf),
            "wk": w_qkv[:, D + c0:D + c0 + HL * DH].astype(bf),
            "wv": w_qkv[:, 2 * D + c0:2 * D + c0 + HL * DH].astype(bf),
            "wo": w_out[c0:c0 + HL * DH, :].astype(bf),
        })
    return in_maps


def run_sharded(x, w_qkv, w_out, trace=False, tmpdir=None):
    nc = build()
    res = run_bass_kernel_spmd(
        nc, make_in_maps(x, w_qkv, w_out), core_ids=list(range(N_CORES)),
        trace=trace, tmpdir=tmpdir,
    )
    out = np.zeros((B, S, D), np.float32)
    for core in range(N_CORES):
        out[core // 4] += res.results[core]["out"].astype(np.float32)
    return out, res.exec_time_ns


def kernel(x, w_qkv, w_out):
    out, _ = run_sharded(x, w_qkv, w_out)
    return out
